# revision 55
# baseline (speedup 1.0000x reference)
"""DeepNCM Trainium2 kernel v3: fp8 DoubleRow one-hot segment sums +
fp8 DoubleRow distance GEMM, data-parallel over embedding rows across 8 cores.

Contract: kernel(**inputs) takes the FULL unsharded inputs
(embeddings [65536,512] f32, prototypes [1000,512] f32, counter [1000] f32,
y_true [65536] int64) and returns the FULL output [65536,1000] f32.

Per-core plan (NL = 8192 rows):
  Host precomputes counts = bincount(y) and folds the running-mean update
  into per-class coefficients: protos2 = A2*p0 + B2*sums (factor 2 folded).
  The host also ships A2*p0^T and broadcast B2 so the device applies them
  with two elementwise ops.
  Phase 1 (per pair of 128-row tiles): DMA f32 emb; quantize to fp8;
  ACT computes e_sq (Square+accum from f32); DVE/Pool build fp8 one-hot
  tiles; PE accumulates sumsT[d,c] += emb^T @ onehot with DoubleRow fp8
  matmuls (two row-tiles per instruction). PSUM sums -> bf16 -> DRAM.
  ReduceScatter gives each core a 64-row D-slice of the reduced sumsT;
  it computes its protos2T slice + a -p_sq/4 partial row, quantizes to
  fp8, and an AllGather replicates [8*(64+1), 1000] fp8 — already in the
  [D, C] layout phase 2 needs. p_sq partials are summed with a tiny
  ones-matmul into the K=1 fold rows.
  Phase 2: cross via fp8 DoubleRow matmuls + a K=1 DoubleRow instruction
  folding -p_sq into PSUM; ACT/DVE epilogue adds -e_sq (per-partition
  bias) and writes fp16; host upcasts. Emb transposes (PE, fp8, stride-2
  PSUM) and their SBUF copies run inside the collective window.
"""

import os
import sys
from contextlib import ExitStack

for _p in ("/opt/trn_rl_repo", "/root/.axon_site/_ro/trn_rl_repo"):
    if os.path.isdir(_p):
        if _p not in sys.path:
            sys.path.insert(0, _p)
        break

import numpy as np

import concourse.bass as bass
import concourse.mybir as mybir
import concourse.tile as tile
from concourse.masks import make_identity
from concourse.bass_utils import run_bass_kernel_spmd

N, D, C = 65536, 512, 1000
W = 8                      # cores
NL = N // W                # rows per core
P = 128
KT = NL // P               # 64 row tiles per core
DC = D // P                # 4 D chunks of 128
DS = D // W                # 64-row D-slice per core after ReduceScatter
AGB = DS + 1               # AllGather block: 64 protos2T rows + 1 psq row
CH = ((0, 512), (512, 1000))   # free-dim halves of the class axis
F32 = mybir.dt.float32
F16 = mybir.dt.float16
BF16 = mybir.dt.bfloat16
FP8 = mybir.dt.float8e4
ALU = mybir.AluOpType
ACTF = mybir.ActivationFunctionType
DR = mybir.MatmulPerfMode.DoubleRow

# Toggled by test.py for profiling runs.
PROFILE = False
TRACE_KWARGS = {}
LAST_RESULT = [None]

_built = [None]


def _split_waits(nc, cap=1):
    """Walrus in this container rejects >1 sync-wait per instruction.
    Move excess waits onto preceding same-engine NOPs (in-order engines,
    so semantics are preserved)."""
    n_new = 0
    for fn in nc.m.functions:
        for bb in fn.blocks:
            new_list = []
            for ins in bb.instructions:
                si = getattr(ins, "sync_info", None)
                if si is not None and si.on_wait and len(si.on_wait) > cap:
                    waits = list(si.on_wait)
                    keep, rest = waits[:cap], waits[cap:]
                    for i in range(0, len(rest), cap):
                        nop = mybir.InstNoOp(
                            name=f"I-waitsplit-{n_new}", ins=[], outs=[]
                        )
                        n_new += 1
                        nop.engine = ins.engine
                        nop.sync_info = mybir.SyncInfo(
                            on_wait=rest[i : i + cap], on_update=[]
                        )
                        new_list.append(nop)
                    si.on_wait = keep
                new_list.append(ins)
            bb.instructions = new_list
    return n_new


def _build():
    nc = bass.Bass()
    emb_ext = nc.declare_dram_parameter("emb", [NL, D], F32, isOutput=False)
    yf_ext = nc.declare_dram_parameter("yf", [P, KT], F32, isOutput=False)
    ap0_ext = nc.declare_dram_parameter("ap0", [DS, C], F32, isOutput=False)
    bb_ext = nc.declare_dram_parameter("bb", [DS, C], F32, isOutput=False)
    out_ext = nc.declare_dram_parameter("out", [NL, C], F16, isOutput=True)

    with tile.TileContext(nc) as tc, ExitStack() as es:
        cpool = es.enter_context(tc.tile_pool(name="const", bufs=1))
        bpool = es.enter_context(tc.tile_pool(name="bigs", bufs=1))
        in_pool = es.enter_context(tc.tile_pool(name="inp", bufs=20))
        oh_pool = es.enter_context(tc.tile_pool(name="oh", bufs=4))
        sq_pool = es.enter_context(tc.tile_pool(name="sq", bufs=2))
        out_pool = es.enter_context(tc.tile_pool(name="outp", bufs=6))
        dram = es.enter_context(tc.tile_pool(name="dram", bufs=1, space="DRAM"))

        # ---- constants ----
        ident_8 = cpool.tile([P, P], FP8, name="ident8")
        make_identity(nc, ident_8[:])
        fold_ones = cpool.tile([1, 2 * P], FP8, name="fones")
        nc.vector.memset(fold_ones[:], 1.0)
        ones8 = cpool.tile([P, 1], FP8, name="ones8")
        nc.vector.memset(ones8[:], 1.0)
        iota = cpool.tile([P, C], F32, name="iota")
        nc.gpsimd.iota(
            iota[:], pattern=[[1, C]], base=0, channel_multiplier=0,
            allow_small_or_imprecise_dtypes=True,
        )
        y_sb = cpool.tile([P, KT], F32, name="y")
        nc.sync.dma_start(y_sb[:], yf_ext[:])
        ap0_sb = cpool.tile([P, C], F32, name="ap0")
        nc.sync.dma_start(ap0_sb[0:DS, :], ap0_ext[:])
        bb_sb = cpool.tile([P, C], F32, name="bb")
        nc.sync.dma_start(bb_sb[0:DS, :], bb_ext[:])

        esq_neg = cpool.tile([P, KT], F32, name="esqn")
        e8 = bpool.tile([P, KT * D], FP8, name="e8")
        embT8 = bpool.tile([P, KT * D], FP8, name="embT8")
        protosT8 = bpool.tile([P, DC * C], FP8, name="protosT8")
        fold_rhs = bpool.tile([1, 2 * C], FP8, name="foldr")
        nc.vector.memset(fold_rhs[0:1, C : 2 * C], 0.0)

        deferred_esq = []
        # ================= phase 1: one-hot segment sums =================
        with tc.tile_pool(name="ps_sums", bufs=1, space="PSUM") as ps_sums:
            s_ps = [
                [ps_sums.tile([P, c1 - c0], F32, tag=f"s{dc}_{ci}",
                              name=f"s{dc}_{ci}")
                 for ci, (c0, c1) in enumerate(CH)]
                for dc in range(DC)
            ]
            for kp in range(KT // 2):
                t0 = kp * 2
                et = in_pool.tile([P, 2 * D], F32, tag="et", name="et")
                src = emb_ext[t0 * P : (t0 + 2) * P, :].rearrange(
                    "(i p) d -> p i d", i=2
                )
                ld_eng = nc.sync if kp % 4 != 1 else nc.gpsimd
                ld_eng.dma_start(et.rearrange("p (i d) -> p i d", i=2), src)
                # fp8 quantization (feeds the sums matmuls AND phase 2)
                dst = e8[:, t0 * D : (t0 + 2) * D]
                if kp % 2 == 0:
                    nc.gpsimd.tensor_copy(out=dst, in_=et[:])
                else:
                    nc.vector.tensor_copy(out=dst, in_=et[:])
                # e_sq from f32 (exact); deferred for the last 16 pairs so
                # ACT can drain the transpose copies first
                if kp < 12:
                    for i in range(2):
                        scr = sq_pool.tile([P, D], BF16, tag="scr", name="scr")
                        nc.scalar.activation(
                            scr[:], et[:, i * D : (i + 1) * D], ACTF.Square,
                            accum_out=esq_neg[:, t0 + i : t0 + i + 1],
                        )
                else:
                    deferred_esq.append((et, t0))
                # one-hot pair tile [128, 2, C] fp8
                oh = oh_pool.tile([P, 2 * C], FP8, tag="oh", name="oh")
                for i in range(2):
                    oh_eng = nc.vector if (kp + i) % 3 != 2 else nc.gpsimd
                    oh_eng.tensor_scalar(
                        oh[:, i * C : (i + 1) * C], iota[:],
                        y_sb[:, t0 + i : t0 + i + 1], None, ALU.is_equal,
                    )
                ohv = oh.rearrange("p (pl c) -> p pl c", pl=2)
                e8v = e8.rearrange("p (nt dc m) -> p nt dc m", nt=KT, dc=DC)
                for dc in range(DC):
                    lhs = e8v[:, t0 : t0 + 2, dc, :]  # [P, 2, 128]
                    for ci, (c0, c1) in enumerate(CH):
                        nc.tensor.matmul(
                            s_ps[dc][ci][:],
                            lhs,
                            ohv[:, :, c0:c1],
                            start=(kp == 0), stop=(kp == KT // 2 - 1),
                            perf_mode=DR,
                        )
            # sums psum -> sbuf bf16 (D-major [512, 1000])
            sums_sb = cpool.tile([P, DC * C], BF16, name="sumssb")
            for dc in range(DC):
                for ci, (c0, c1) in enumerate(CH):
                    dsts = sums_sb[:, dc * C + c0 : dc * C + c1]
                    if (dc + ci) % 2 == 0:
                        nc.scalar.copy(dsts, s_ps[dc][ci][:])
                    else:
                        nc.vector.tensor_copy(out=dsts, in_=s_ps[dc][ci][:])

        sums_d = dram.tile([D, C], BF16, name="sumsd")
        for dc in range(DC):
            (nc.sync if dc % 2 == 0 else nc.gpsimd).dma_start(
                sums_d[dc * P : (dc + 1) * P, :],
                sums_sb[:, dc * C : (dc + 1) * C],
            )

        # ---- ReduceScatter: core i owns D rows [64i, 64i+64) ----
        rs_out = dram.tile([DS, C], BF16, name="rsout")
        nc.gpsimd.collective_compute(
            "ReduceScatter", ALU.add,
            replica_groups=[list(range(W))],
            ins=[sums_d.opt()], outs=[rs_out.opt()],
        )
        sums_rs = cpool.tile([P, C], BF16, name="sumsrs")
        nc.sync.dma_start(sums_rs[0:DS, :], rs_out[:])

        # ---- protos2T slice + psq partial, quantize, AllGather ----
        pr2 = cpool.tile([P, C], FP8, name="pr2")
        t2 = cpool.tile([P, C], F32, name="t2")
        sq8 = cpool.tile([P, C], FP8, name="sq8")
        for c0, c1 in CH:
            nc.vector.tensor_tensor(out=t2[0:DS, c0:c1],
                                    in0=sums_rs[0:DS, c0:c1],
                                    in1=bb_sb[0:DS, c0:c1], op=ALU.mult)
            nc.vector.tensor_tensor(out=pr2[0:DS, c0:c1],
                                    in0=t2[0:DS, c0:c1],
                                    in1=ap0_sb[0:DS, c0:c1], op=ALU.add)
            nc.vector.tensor_tensor(out=sq8[0:DS, c0:c1],
                                    in0=pr2[0:DS, c0:c1],
                                    in1=pr2[0:DS, c0:c1], op=ALU.mult)

        ag_in = dram.tile([AGB, C], FP8, name="agin")
        ag_out = dram.tile([W * AGB, C], FP8, name="agout",
                           addr_space="Shared")

        def _psq_ag():
            psq8 = cpool.tile([1, C], FP8, name="psq8")
            with tc.tile_pool(name="ps_pq", bufs=1, space="PSUM") as ps_pq:
                for ci, (c0, c1) in enumerate(CH):
                    pq = ps_pq.tile([1, c1 - c0], F32, tag=f"pq{ci}",
                                    name=f"pq{ci}")
                    nc.tensor.matmul(pq[:], ones8[0:DS, :], sq8[0:DS, c0:c1],
                                     start=True, stop=True)
                    nc.vector.tensor_scalar(psq8[0:1, c0:c1], pq[:],
                                            -0.25, None, ALU.mult)

            nc.sync.dma_start(ag_in[0:DS, :], pr2[0:DS, :])
            nc.sync.dma_start(ag_in[DS : DS + 1, :], psq8[:])
            nc.gpsimd.collective_compute(
                "AllGather", ALU.bypass,
                replica_groups=[list(range(W))],
                ins=[ag_in.opt()], outs=[ag_out.opt()],
            )

        # ---- emb transposes (fp8, stride-2 psum) fill the collective gap ----
        with tc.tile_pool(name="ps_tr", bufs=6, space="PSUM") as ps_tr:
            for t in range(KT):
                if t == 44:
                    _psq_ag()
                trb = ps_tr.tile([P, 2 * D], FP8, tag="trb", name="trb")
                trv = trb.rearrange("p (c two) -> p c two", two=2)
                for dc in range(DC):
                    nc.tensor.matmul(
                        trv[:, dc * P : (dc + 1) * P, 0:1],
                        e8[:, t * D + dc * P : t * D + (dc + 1) * P],
                        ident_8[:],
                        is_transpose=True,
                        start=(dc == 0), stop=(dc == DC - 1),
                    )
                dst8 = embT8[:, t * D : (t + 1) * D]
                nc.scalar.copy(dst8, trv[:, 0 : D, 0])


        # psq partial rows first (critical path to the fold rows)
        psqs = cpool.tile([8, C], FP8, name="psqs")
        nc.sync.dma_start(
            psqs[:],
            ag_out.rearrange("(k b) c -> k b c", b=AGB)[:, DS, :],
        )
        # protos2T blocks land pre-transposed: block k rows -> chunk layout
        agov = ag_out.rearrange("(dcq h b) c -> dcq h b c", dcq=DC, h=2)
        ptv = protosT8.rearrange("p (dcq c) -> p dcq c", dcq=DC)
        for h in range(2):
            (nc.scalar if h == 0 else nc.gpsimd).dma_start(
                ptv[h * DS : (h + 1) * DS, :, :],
                agov[:, h, 0:DS, :].rearrange("dcq b c -> b dcq c"),
            )
        with tc.tile_pool(name="ps_pf", bufs=1, space="PSUM") as ps_pf:
            for ci, (c0, c1) in enumerate(CH):
                pf = ps_pf.tile([1, c1 - c0], F32, tag=f"pf{ci}",
                                name=f"pf{ci}")
                nc.tensor.matmul(pf[:], ones8[0:8, :], psqs[:, c0:c1],
                                 start=True, stop=True)
                nc.vector.tensor_copy(out=fold_rhs[0:1, c0:c1],
                                      in_=pf[:])

        for et_d, t0 in deferred_esq:
            for i in range(2):
                scr = sq_pool.tile([P, D], BF16, tag="scr", name="scr")
                nc.scalar.activation(
                    scr[:], et_d[:, i * D : (i + 1) * D], ACTF.Square,
                    accum_out=esq_neg[:, t0 + i : t0 + i + 1],
                )

        # negate e_sq once (used as ScalarE bias in phase 2)
        nc.vector.tensor_scalar(esq_neg[:], esq_neg[:], -1.0, None, ALU.mult)

        # ================= phase 2 =================
        fones_v = fold_ones.rearrange("p (pl m) -> p pl m", pl=2)
        frhs_v = fold_rhs.rearrange("p (pl c) -> p pl c", pl=2)
        with tc.tile_pool(name="ps_cr", bufs=4, space="PSUM") as ps_cr:
            for nt in range(KT):
                ot = out_pool.tile([P, C], F16, tag="ot", name="ot")
                for ci, (c0, c1) in enumerate(CH):
                    cr = ps_cr.tile([P, c1 - c0], F32, tag=f"cr{ci}",
                                    name=f"cr{ci}")
                    nc.tensor.matmul(
                        cr[:], fones_v[:, :, :], frhs_v[:, :, c0:c1],
                        start=True, stop=False, perf_mode=DR,
                    )
                    for pr in range(2):
                        lhs = embT8[
                            :, nt * D + pr * 2 * P : nt * D + (pr + 1) * 2 * P
                        ].rearrange("p (pl m) -> p pl m", pl=2)
                        rhs = protosT8[
                            :, 2 * pr * C : (2 * pr + 2) * C
                        ].rearrange("p (pl c) -> p pl c", pl=2)[:, :, c0:c1]
                        nc.tensor.matmul(
                            cr[:], lhs, rhs,
                            start=False, stop=(pr == 1),
                            perf_mode=DR,
                        )
                    if (2 * nt + ci) % 2 == 0:
                        nc.scalar.activation(
                            ot[:, c0:c1], cr[:], ACTF.Identity,
                            bias=esq_neg[:, nt : nt + 1], scale=1.0,
                        )
                    else:
                        nc.vector.tensor_scalar(
                            ot[:, c0:c1], cr[:], esq_neg[:, nt : nt + 1],
                            None, ALU.add,
                        )
                st_eng = nc.sync if nt % 3 < 2 else nc.gpsimd
                st_eng.dma_start(out_ext[nt * P : (nt + 1) * P, :], ot[:])

    _split_waits(nc)
    return nc


def kernel(embeddings, prototypes, counter, y_true):
    embeddings = np.ascontiguousarray(np.asarray(embeddings, dtype=np.float32))
    prototypes = np.ascontiguousarray(np.asarray(prototypes, dtype=np.float32))
    counter_f = np.asarray(counter, dtype=np.float64)
    y = np.asarray(y_true).astype(np.int64)

    # host-side: counts + running-mean coefficients (index math only)
    counts = np.bincount(y, minlength=C).astype(np.float64)
    rep = counts > 0
    rm = 1.0 / np.maximum(counts, 1.0)
    rt = 1.0 / (counter_f + 1.0)
    B2 = (2.0 * rep * rm * rt).astype(np.float32)
    A2 = (2.0 * (1.0 + rep * (counter_f * rt - 1.0))).astype(np.float32)
    p0T = prototypes.T  # [D, C]

    if _built[0] is None:
        _built[0] = _build()
    nc = _built[0]

    in_maps = []
    for i in range(W):
        sl = slice(i * NL, (i + 1) * NL)
        ds = slice(i * DS, (i + 1) * DS)
        y_loc = y[sl].astype(np.float32)
        yf = np.ascontiguousarray(y_loc.reshape(KT, P).T)
        in_maps.append(
            {
                "emb": embeddings[sl],
                "yf": yf,
                "ap0": np.ascontiguousarray(A2[None, :] * p0T[ds]),
                "bb": np.ascontiguousarray(
                    np.broadcast_to(B2[None, :], (DS, C))
                ),
            }
        )

    res = run_bass_kernel_spmd(
        nc, in_maps, list(range(W)), trace=PROFILE, **TRACE_KWARGS
    )
    LAST_RESULT[0] = res
    out = np.concatenate([res.results[i]["out"] for i in range(W)], axis=0)
    return out.astype(np.float32)


# revision 58
# speedup vs baseline: 1.0238x; 1.0238x over previous
"""DeepNCM Trainium2 kernel v3: fp8 DoubleRow one-hot segment sums +
fp8 DoubleRow distance GEMM, data-parallel over embedding rows across 8 cores.

Contract: kernel(**inputs) takes the FULL unsharded inputs
(embeddings [65536,512] f32, prototypes [1000,512] f32, counter [1000] f32,
y_true [65536] int64) and returns the FULL output [65536,1000] f32.

Per-core plan (NL = 8192 rows):
  Host precomputes counts = bincount(y) and folds the running-mean update
  into per-class coefficients: protos2 = A2*p0 + B2*sums (factor 2 folded).
  The host also ships A2*p0^T and broadcast B2 so the device applies them
  with two elementwise ops.
  Phase 1 (per pair of 128-row tiles): DMA f32 emb; quantize to fp8;
  ACT computes e_sq (Square+accum from f32); DVE/Pool build fp8 one-hot
  tiles; PE accumulates sumsT[d,c] += emb^T @ onehot with DoubleRow fp8
  matmuls (two row-tiles per instruction). PSUM sums -> bf16 -> DRAM.
  ReduceScatter gives each core a 64-row D-slice of the reduced sumsT;
  it computes its protos2T slice + a -p_sq/4 partial row, quantizes to
  fp8, and an AllGather replicates [8*(64+1), 1000] fp8 — already in the
  [D, C] layout phase 2 needs. p_sq partials are summed with a tiny
  ones-matmul into the K=1 fold rows.
  Phase 2: cross via fp8 DoubleRow matmuls + a K=1 DoubleRow instruction
  folding -p_sq into PSUM; ACT/DVE epilogue adds -e_sq (per-partition
  bias) and writes fp16; host upcasts. Emb transposes (PE, fp8, stride-2
  PSUM) and their SBUF copies run inside the collective window.
"""

import os
import sys
from contextlib import ExitStack

for _p in ("/opt/trn_rl_repo", "/root/.axon_site/_ro/trn_rl_repo"):
    if os.path.isdir(_p):
        if _p not in sys.path:
            sys.path.insert(0, _p)
        break

import numpy as np

import concourse.bass as bass
import concourse.mybir as mybir
import concourse.tile as tile
from concourse.masks import make_identity
from concourse.bass_utils import run_bass_kernel_spmd

N, D, C = 65536, 512, 1000
W = 8                      # cores
NL = N // W                # rows per core
P = 128
KT = NL // P               # 64 row tiles per core
DC = D // P                # 4 D chunks of 128
DS = D // W                # 64-row D-slice per core after ReduceScatter
AGB = DS + 1               # AllGather block: 64 protos2T rows + 1 psq row
CH = ((0, 512), (512, 1000))   # free-dim halves of the class axis
F32 = mybir.dt.float32
F16 = mybir.dt.float16
BF16 = mybir.dt.bfloat16
FP8 = mybir.dt.float8e4
ALU = mybir.AluOpType
ACTF = mybir.ActivationFunctionType
DR = mybir.MatmulPerfMode.DoubleRow

# Toggled by test.py for profiling runs.
PROFILE = False
TRACE_KWARGS = {}
LAST_RESULT = [None]

_built = [None]


def _split_waits(nc, cap=1):
    """Walrus in this container rejects >1 sync-wait per instruction.
    Move excess waits onto preceding same-engine NOPs (in-order engines,
    so semantics are preserved)."""
    n_new = 0
    for fn in nc.m.functions:
        for bb in fn.blocks:
            new_list = []
            for ins in bb.instructions:
                si = getattr(ins, "sync_info", None)
                if si is not None and si.on_wait and len(si.on_wait) > cap:
                    waits = list(si.on_wait)
                    keep, rest = waits[:cap], waits[cap:]
                    for i in range(0, len(rest), cap):
                        nop = mybir.InstNoOp(
                            name=f"I-waitsplit-{n_new}", ins=[], outs=[]
                        )
                        n_new += 1
                        nop.engine = ins.engine
                        nop.sync_info = mybir.SyncInfo(
                            on_wait=rest[i : i + cap], on_update=[]
                        )
                        new_list.append(nop)
                    si.on_wait = keep
                new_list.append(ins)
            bb.instructions = new_list
    return n_new


def _build():
    nc = bass.Bass()
    emb_ext = nc.declare_dram_parameter("emb", [NL, D], F32, isOutput=False)
    yf_ext = nc.declare_dram_parameter("yf", [P, KT], F32, isOutput=False)
    ap0_ext = nc.declare_dram_parameter("ap0", [DS, C], F32, isOutput=False)
    bb_ext = nc.declare_dram_parameter("bb", [DS, C], F32, isOutput=False)
    out_ext = nc.declare_dram_parameter("out", [NL, C], F16, isOutput=True)

    with tile.TileContext(nc) as tc, ExitStack() as es:
        cpool = es.enter_context(tc.tile_pool(name="const", bufs=1))
        bpool = es.enter_context(tc.tile_pool(name="bigs", bufs=1))
        in_pool = es.enter_context(tc.tile_pool(name="inp", bufs=20))
        oh_pool = es.enter_context(tc.tile_pool(name="oh", bufs=4))
        sq_pool = es.enter_context(tc.tile_pool(name="sq", bufs=2))
        out_pool = es.enter_context(tc.tile_pool(name="outp", bufs=6))
        dram = es.enter_context(tc.tile_pool(name="dram", bufs=1, space="DRAM"))

        # ---- constants ----
        ident_8 = cpool.tile([P, P], FP8, name="ident8")
        make_identity(nc, ident_8[:])
        fold_ones = cpool.tile([1, 2 * P], FP8, name="fones")
        nc.vector.memset(fold_ones[:], 1.0)
        ones8 = cpool.tile([P, 1], FP8, name="ones8")
        nc.vector.memset(ones8[:], 1.0)
        iota = cpool.tile([P, C], F32, name="iota")
        nc.gpsimd.iota(
            iota[:], pattern=[[1, C]], base=0, channel_multiplier=0,
            allow_small_or_imprecise_dtypes=True,
        )
        y_sb = cpool.tile([P, KT], F32, name="y")
        nc.sync.dma_start(y_sb[:], yf_ext[:])
        ap0_sb = cpool.tile([P, C], F32, name="ap0")
        nc.scalar.dma_start(ap0_sb[0:DS, :], ap0_ext[:])
        bb_sb = cpool.tile([P, C], F32, name="bb")
        nc.scalar.dma_start(bb_sb[0:DS, :], bb_ext[:])

        esq_neg = cpool.tile([P, KT], F32, name="esqn")
        e8 = bpool.tile([P, KT * D], FP8, name="e8")
        embT8 = bpool.tile([P, KT * D], FP8, name="embT8")
        protosT8 = bpool.tile([P, DC * C], FP8, name="protosT8")
        fold_rhs = bpool.tile([1, 2 * C], FP8, name="foldr")
        nc.vector.memset(fold_rhs[0:1, C : 2 * C], 0.0)

        deferred_esq = []
        # ================= phase 1: one-hot segment sums =================
        with tc.tile_pool(name="ps_sums", bufs=1, space="PSUM") as ps_sums:
            s_ps = [
                [ps_sums.tile([P, c1 - c0], F32, tag=f"s{dc}_{ci}",
                              name=f"s{dc}_{ci}")
                 for ci, (c0, c1) in enumerate(CH)]
                for dc in range(DC)
            ]
            for kp in range(KT // 2):
                t0 = kp * 2
                et = in_pool.tile([P, 2 * D], F32, tag="et", name="et")
                src = emb_ext[t0 * P : (t0 + 2) * P, :].rearrange(
                    "(i p) d -> p i d", i=2
                )
                ld_eng = nc.sync if kp % 4 != 1 else nc.gpsimd
                ld_eng.dma_start(et.rearrange("p (i d) -> p i d", i=2), src)
                # fp8 quantization (feeds the sums matmuls AND phase 2)
                dst = e8[:, t0 * D : (t0 + 2) * D]
                if kp % 2 == 0:
                    nc.gpsimd.tensor_copy(out=dst, in_=et[:])
                else:
                    nc.vector.tensor_copy(out=dst, in_=et[:])
                # e_sq from f32 (exact); deferred for the last 16 pairs so
                # ACT can drain the transpose copies first
                if kp < 12:
                    for i in range(2):
                        scr = sq_pool.tile([P, D], BF16, tag="scr", name="scr")
                        nc.scalar.activation(
                            scr[:], et[:, i * D : (i + 1) * D], ACTF.Square,
                            accum_out=esq_neg[:, t0 + i : t0 + i + 1],
                        )
                else:
                    deferred_esq.append((et, t0))
                # one-hot pair tile [128, 2, C] fp8
                oh = oh_pool.tile([P, 2 * C], FP8, tag="oh", name="oh")
                for i in range(2):
                    oh_eng = nc.vector if (kp + i) % 3 != 2 else nc.gpsimd
                    oh_eng.tensor_scalar(
                        oh[:, i * C : (i + 1) * C], iota[:],
                        y_sb[:, t0 + i : t0 + i + 1], None, ALU.is_equal,
                    )
                ohv = oh.rearrange("p (pl c) -> p pl c", pl=2)
                e8v = e8.rearrange("p (nt dc m) -> p nt dc m", nt=KT, dc=DC)
                for dc in range(DC):
                    lhs = e8v[:, t0 : t0 + 2, dc, :]  # [P, 2, 128]
                    for ci, (c0, c1) in enumerate(CH):
                        nc.tensor.matmul(
                            s_ps[dc][ci][:],
                            lhs,
                            ohv[:, :, c0:c1],
                            start=(kp == 0), stop=(kp == KT // 2 - 1),
                            perf_mode=DR,
                        )
            # sums psum -> sbuf bf16 (D-major [512, 1000])
            sums_sb = cpool.tile([P, DC * C], BF16, name="sumssb")
            for dc in range(DC):
                for ci, (c0, c1) in enumerate(CH):
                    dsts = sums_sb[:, dc * C + c0 : dc * C + c1]
                    if (dc + ci) % 2 == 0:
                        nc.scalar.copy(dsts, s_ps[dc][ci][:])
                    else:
                        nc.vector.tensor_copy(out=dsts, in_=s_ps[dc][ci][:])

        sums_d = dram.tile([D, C], BF16, name="sumsd")
        for dc in range(DC):
            (nc.sync if dc % 2 == 0 else nc.gpsimd).dma_start(
                sums_d[dc * P : (dc + 1) * P, :],
                sums_sb[:, dc * C : (dc + 1) * C],
            )

        # ---- ReduceScatter: core i owns D rows [64i, 64i+64) ----
        rs_out = dram.tile([DS, C], BF16, name="rsout")
        nc.gpsimd.collective_compute(
            "ReduceScatter", ALU.add,
            replica_groups=[list(range(W))],
            ins=[sums_d.opt()], outs=[rs_out.opt()],
        )
        sums_rs = cpool.tile([P, C], BF16, name="sumsrs")
        nc.sync.dma_start(sums_rs[0:DS, :], rs_out[:])

        # ---- protos2T slice + psq partial, quantize, AllGather ----
        pr2 = cpool.tile([P, C], FP8, name="pr2")
        t2 = cpool.tile([P, C], F32, name="t2")
        sq8 = cpool.tile([P, C], FP8, name="sq8")
        for c0, c1 in CH:
            nc.vector.tensor_tensor(out=t2[0:DS, c0:c1],
                                    in0=sums_rs[0:DS, c0:c1],
                                    in1=bb_sb[0:DS, c0:c1], op=ALU.mult)
            nc.vector.tensor_tensor(out=pr2[0:DS, c0:c1],
                                    in0=t2[0:DS, c0:c1],
                                    in1=ap0_sb[0:DS, c0:c1], op=ALU.add)
            nc.vector.tensor_tensor(out=sq8[0:DS, c0:c1],
                                    in0=pr2[0:DS, c0:c1],
                                    in1=pr2[0:DS, c0:c1], op=ALU.mult)

        ag_in = dram.tile([AGB, C], FP8, name="agin")
        ag_out = dram.tile([W * AGB, C], FP8, name="agout",
                           addr_space="Shared")

        def _psq_ag():
            psq8 = cpool.tile([1, C], FP8, name="psq8")
            with tc.tile_pool(name="ps_pq", bufs=1, space="PSUM") as ps_pq:
                for ci, (c0, c1) in enumerate(CH):
                    pq = ps_pq.tile([1, c1 - c0], F32, tag=f"pq{ci}",
                                    name=f"pq{ci}")
                    nc.tensor.matmul(pq[:], ones8[0:DS, :], sq8[0:DS, c0:c1],
                                     start=True, stop=True)
                    nc.vector.tensor_scalar(psq8[0:1, c0:c1], pq[:],
                                            -0.25, None, ALU.mult)

            nc.sync.dma_start(ag_in[0:DS, :], pr2[0:DS, :])
            nc.sync.dma_start(ag_in[DS : DS + 1, :], psq8[:])
            nc.gpsimd.collective_compute(
                "AllGather", ALU.bypass,
                replica_groups=[list(range(W))],
                ins=[ag_in.opt()], outs=[ag_out.opt()],
            )

        # ---- emb transposes (fp8, stride-2 psum) fill the collective gap ----
        with tc.tile_pool(name="ps_tr", bufs=6, space="PSUM") as ps_tr:
            for t in range(KT):
                if t == 44:
                    _psq_ag()
                trb = ps_tr.tile([P, 2 * D], FP8, tag="trb", name="trb")
                trv = trb.rearrange("p (c two) -> p c two", two=2)
                for dc in range(DC):
                    nc.tensor.matmul(
                        trv[:, dc * P : (dc + 1) * P, 0:1],
                        e8[:, t * D + dc * P : t * D + (dc + 1) * P],
                        ident_8[:],
                        is_transpose=True,
                        start=(dc == 0), stop=(dc == DC - 1),
                    )
                dst8 = embT8[:, t * D : (t + 1) * D]
                nc.scalar.copy(dst8, trv[:, 0 : D, 0])


        # psq partial rows first (critical path to the fold rows)
        psqs = cpool.tile([8, C], FP8, name="psqs")
        nc.sync.dma_start(
            psqs[:],
            ag_out.rearrange("(k b) c -> k b c", b=AGB)[:, DS, :],
        )
        # protos2T blocks land pre-transposed: block k rows -> chunk layout
        agov = ag_out.rearrange("(dcq h b) c -> dcq h b c", dcq=DC, h=2)
        ptv = protosT8.rearrange("p (dcq c) -> p dcq c", dcq=DC)
        for h in range(2):
            (nc.scalar if h == 0 else nc.gpsimd).dma_start(
                ptv[h * DS : (h + 1) * DS, :, :],
                agov[:, h, 0:DS, :].rearrange("dcq b c -> b dcq c"),
            )
        with tc.tile_pool(name="ps_pf", bufs=1, space="PSUM") as ps_pf:
            for ci, (c0, c1) in enumerate(CH):
                pf = ps_pf.tile([1, c1 - c0], F32, tag=f"pf{ci}",
                                name=f"pf{ci}")
                nc.tensor.matmul(pf[:], ones8[0:8, :], psqs[:, c0:c1],
                                 start=True, stop=True)
                nc.vector.tensor_copy(out=fold_rhs[0:1, c0:c1],
                                      in_=pf[:])

        for et_d, t0 in deferred_esq:
            for i in range(2):
                scr = sq_pool.tile([P, D], BF16, tag="scr", name="scr")
                nc.scalar.activation(
                    scr[:], et_d[:, i * D : (i + 1) * D], ACTF.Square,
                    accum_out=esq_neg[:, t0 + i : t0 + i + 1],
                )

        # negate e_sq once (used as ScalarE bias in phase 2)
        nc.vector.tensor_scalar(esq_neg[:], esq_neg[:], -1.0, None, ALU.mult)

        # ================= phase 2 =================
        fones_v = fold_ones.rearrange("p (pl m) -> p pl m", pl=2)
        frhs_v = fold_rhs.rearrange("p (pl c) -> p pl c", pl=2)
        with tc.tile_pool(name="ps_cr", bufs=4, space="PSUM") as ps_cr:
            for nt in range(KT):
                ot = out_pool.tile([P, C], F16, tag="ot", name="ot")
                for ci, (c0, c1) in enumerate(CH):
                    cr = ps_cr.tile([P, c1 - c0], F32, tag=f"cr{ci}",
                                    name=f"cr{ci}")
                    nc.tensor.matmul(
                        cr[:], fones_v[:, :, :], frhs_v[:, :, c0:c1],
                        start=True, stop=False, perf_mode=DR,
                    )
                    for pr in range(2):
                        lhs = embT8[
                            :, nt * D + pr * 2 * P : nt * D + (pr + 1) * 2 * P
                        ].rearrange("p (pl m) -> p pl m", pl=2)
                        rhs = protosT8[
                            :, 2 * pr * C : (2 * pr + 2) * C
                        ].rearrange("p (pl c) -> p pl c", pl=2)[:, :, c0:c1]
                        nc.tensor.matmul(
                            cr[:], lhs, rhs,
                            start=False, stop=(pr == 1),
                            perf_mode=DR,
                        )
                    if (2 * nt + ci) % 2 == 0:
                        nc.scalar.activation(
                            ot[:, c0:c1], cr[:], ACTF.Identity,
                            bias=esq_neg[:, nt : nt + 1], scale=1.0,
                        )
                    else:
                        nc.vector.tensor_scalar(
                            ot[:, c0:c1], cr[:], esq_neg[:, nt : nt + 1],
                            None, ALU.add,
                        )
                st_eng = nc.sync if nt % 3 < 2 else nc.gpsimd
                st_eng.dma_start(out_ext[nt * P : (nt + 1) * P, :], ot[:])

    _split_waits(nc)
    return nc


def kernel(embeddings, prototypes, counter, y_true):
    embeddings = np.ascontiguousarray(np.asarray(embeddings, dtype=np.float32))
    prototypes = np.ascontiguousarray(np.asarray(prototypes, dtype=np.float32))
    counter_f = np.asarray(counter, dtype=np.float64)
    y = np.asarray(y_true).astype(np.int64)

    # host-side: counts + running-mean coefficients (index math only)
    counts = np.bincount(y, minlength=C).astype(np.float64)
    rep = counts > 0
    rm = 1.0 / np.maximum(counts, 1.0)
    rt = 1.0 / (counter_f + 1.0)
    B2 = (2.0 * rep * rm * rt).astype(np.float32)
    A2 = (2.0 * (1.0 + rep * (counter_f * rt - 1.0))).astype(np.float32)
    p0T = prototypes.T  # [D, C]

    if _built[0] is None:
        _built[0] = _build()
    nc = _built[0]

    in_maps = []
    for i in range(W):
        sl = slice(i * NL, (i + 1) * NL)
        ds = slice(i * DS, (i + 1) * DS)
        y_loc = y[sl].astype(np.float32)
        yf = np.ascontiguousarray(y_loc.reshape(KT, P).T)
        in_maps.append(
            {
                "emb": embeddings[sl],
                "yf": yf,
                "ap0": np.ascontiguousarray(A2[None, :] * p0T[ds]),
                "bb": np.ascontiguousarray(
                    np.broadcast_to(B2[None, :], (DS, C))
                ),
            }
        )

    res = run_bass_kernel_spmd(
        nc, in_maps, list(range(W)), trace=PROFILE, **TRACE_KWARGS
    )
    LAST_RESULT[0] = res
    out = np.concatenate([res.results[i]["out"] for i in range(W)], axis=0)
    return out.astype(np.float32)
